# revision 9
# baseline (speedup 1.0000x reference)
"""Causal self-attention Trainium2 Bass kernel.

Problem: B=4, T=2048, E=1024, H=16 heads, D=64. fp32 in/out.
Sharding: 8 cores = 4 batches x 2 head-groups (8 heads each).
Per-core pipeline (all matmuls in float32r = full-speed fp32):
  QKV projections -> Q^T,K^T [c,t] + V [t,c] on SBUF
  per head: S^T = K Q^T (row-packed head pairs on the PE array)
  exp via ACT (scale=1/sqrt(D) fused), causal band masked on DVE
  PV with ones-augmented V -> [out^T | rowsum] accumulated in PSUM
  normalize via reciprocal + ones-matmul partition broadcast
  output projection -> partial [T, E] fp32, summed pairwise on host.
"""

import sys

sys.path.insert(0, "/opt/trn_rl_repo")

import numpy as np

import concourse.bass as bass
import concourse.tile as tile
from concourse import bacc, mybir
from concourse.bass import ds, ts
from concourse.bass_utils import run_bass_kernel_spmd

F32 = mybir.dt.float32
F32R = mybir.dt.float32r
AF = mybir.ActivationFunctionType

P = 128
E = 1024
HEADS_PER_CORE = 8
C = HEADS_PER_CORE * 64  # 512 head-dims per core
EO = E // P  # 8
CO = C // P  # 4
TQ = 512  # q tile width
N_CORES = 8


def r(ap):
    return ap


def build_nc(T=2048, reps=1):
    NT = T // TQ  # q/t tiles of 512
    nc = bacc.Bacc("TRN2", target_bir_lowering=False, debug=False)

    xt_d = nc.dram_tensor("xt", [P, EO, T], F32R, kind="ExternalInput").ap()
    wq_d = nc.dram_tensor("wq", [P, EO, C], F32R, kind="ExternalInput").ap()
    wk_d = nc.dram_tensor("wk", [P, EO, C], F32R, kind="ExternalInput").ap()
    wv_d = nc.dram_tensor("wv", [P, EO, C], F32R, kind="ExternalInput").ap()
    wo_d = nc.dram_tensor("wo", [P, CO, E], F32R, kind="ExternalInput").ap()
    bq_d = nc.dram_tensor("bq", [P, CO], F32, kind="ExternalInput").ap()
    bk_d = nc.dram_tensor("bk", [P, CO], F32, kind="ExternalInput").ap()
    bvb_d = nc.dram_tensor("bvb", [P, C], F32R, kind="ExternalInput").ap()
    mk_d = nc.dram_tensor("masks", [P, 2, 1024], F32R, kind="ExternalInput").ap()
    on_d = nc.dram_tensor("onesc", [P, 128], F32R, kind="ExternalInput").ap()
    out_d = nc.dram_tensor("out", [T, E], F32, kind="ExternalOutput").ap()

    with tile.TileContext(nc) as tc, \
         tc.tile_pool(name="psc", bufs=2, space="PSUM") as psc, \
         tc.tile_pool(name="ppv", bufs=3, space="PSUM") as ppv, \
         tc.tile_pool(name="pbc", bufs=1, space="PSUM") as pbc:
        # PSUM pools: 2*2 + 3 + 1 = 8 banks exactly
        for _ in range(reps):
            with tc.tile_pool(name="pers", bufs=1) as pers:
                QT = pers.tile([P, CO, T], F32R, tag="qt")
                KT = pers.tile([P, CO, T], F32R, tag="kt")
                V65 = pers.tile([P, T // P, 520], F32R, tag="v65")
                ones = pers.tile([P, 64], F32R, tag="ones")
                nc.sync.dma_start(ones[:], on_d[:, 0:64])
                # ones column of each V-block
                v_ones = V65.rearrange("p k (h w) -> p k h w", w=65)[:, :, :, 64]
                nc.sync.dma_start(
                    v_ones,
                    on_d[:, : (T // P) * 8].rearrange("p (a b) -> p a b", b=8),
                )

                # ---------------- phase 1: QKV projections ----------------
                with (
                    tc.tile_pool(name="ph1w", bufs=1) as ph1w,
                    tc.tile_pool(name="ph1x", bufs=2) as ph1x,
                ):
                    wq_sb = ph1w.tile([P, EO, C], F32R, tag="wq")
                    wk_sb = ph1w.tile([P, EO, C], F32R, tag="wk")
                    wv_sb = ph1w.tile([P, EO, C], F32R, tag="wv")
                    bq_sb = ph1w.tile([P, CO], F32, tag="bq")
                    bk_sb = ph1w.tile([P, CO], F32, tag="bk")
                    bvb_sb = ph1w.tile([P, C], F32R, tag="bvb")
                    nc.sync.dma_start(wq_sb[:], wq_d)
                    nc.sync.dma_start(wk_sb[:], wk_d)
                    nc.sync.dma_start(wv_sb[:], wv_d)
                    nc.sync.dma_start(bq_sb[:], bq_d)
                    nc.sync.dma_start(bk_sb[:], bk_d)
                    nc.sync.dma_start(bvb_sb[:], bvb_d)

                    for ti in range(NT):
                        xt_t = ph1x.tile([P, EO, TQ], F32R, tag="xt")
                        nc.sync.dma_start(xt_t[:], xt_d[:, :, ts(ti, TQ)])
                        # Q^T and K^T tiles [c=128, t=512]
                        for dst, w_sb, b_sb in (
                            (QT, wq_sb, bq_sb),
                            (KT, wk_sb, bk_sb),
                        ):
                            for co in range(CO):
                                pt = ppv.tile([P, TQ], F32, tag="pv")
                                for eo in range(EO):
                                    nc.tensor.matmul(
                                        pt[:],
                                        r(w_sb[:, eo, ts(co, P)]),
                                        r(xt_t[:, eo, :]),
                                        start=(eo == 0),
                                        stop=(eo == EO - 1),
                                    )
                                nc.scalar.activation(
                                    dst[:, co, ts(ti, TQ)],
                                    pt[:],
                                    AF.Identity,
                                    bias=b_sb[:, co : co + 1],
                                    scale=1.0,
                                )
                        # V tiles [t=128, c=512] -> V65 blocks + bias
                        for tsub in range(TQ // P):
                            kt_i = ti * (TQ // P) + tsub
                            pt = ppv.tile([P, TQ], F32, tag="pv")
                            for eo in range(EO):
                                nc.tensor.matmul(
                                    pt[:, :C],
                                    r(xt_t[:, eo, ts(tsub, P)]),
                                    r(wv_sb[:, eo, :]),
                                    start=(eo == 0),
                                    stop=(eo == EO - 1),
                                )
                            v_dst = V65[:, kt_i, :].rearrange(
                                "p (h w) -> p h w", w=65
                            )[:, :, 0:64]
                            nc.vector.tensor_add(
                                v_dst,
                                pt[:, :C].rearrange("p (h w) -> p h w", w=64),
                                bvb_sb.rearrange("p (h w) -> p h w", w=64),
                            )

                # -------- phase 2: attention + output projection --------
                with (
                    tc.tile_pool(name="ph2", bufs=1) as ph2,
                    tc.tile_pool(name="ppr", bufs=6) as ppr,
                    tc.tile_pool(name="psm", bufs=2) as psm,
                ):
                    wo_sb = ph2.tile([P, CO, E], F32R, tag="wo")
                    mk_sb = ph2.tile([P, 2, 1024], F32R, tag="mk")
                    attn = ph2.tile([P, CO, T], F32R, tag="attn")
                    nc.sync.dma_start(wo_sb[:], wo_d)
                    nc.sync.dma_start(mk_sb[:], mk_d)

                    for qi in range(NT):
                        nkt = 4 * qi + 4  # causal k-tiles of 128
                        ng = nkt // 2  # groups of 2 k-tiles
                        for hp in range(4):  # head pairs
                            po = {}
                            po[0] = ppv.tile([P, TQ], F32, tag="pv", name="po0")
                            po[1] = ppv.tile([P, TQ], F32, tag="pv", name="po1")
                            for g in range(ng):
                                ps = {}
                                pr = {}
                                for s in (0, 1):  # head 2hp+s
                                    ps[s] = psc.tile([P, 1024], F32, tag="sc", name="ps")
                                    lo, hi = 64 * s, 64 * s + 64
                                    for u in (0, 1):
                                        kt_i = 2 * g + u
                                        nc.tensor.matmul(
                                            ps[s][:, ts(u, TQ)],
                                            r(KT[lo:hi, hp, ds(kt_i * P, P)]),
                                            r(QT[lo:hi, hp, ts(qi, TQ)]),
                                            start=True,
                                            stop=True,
                                        )
                                    pr[s] = ppr.tile([P, 1024], F32R, tag="pr", name="pr")
                                    nc.scalar.activation(
                                        pr[s][:], ps[s][:], AF.Exp, scale=0.125
                                    )
                                    if g >= ng - 2:
                                        mg = g - (ng - 2)
                                        nc.vector.tensor_mul(
                                            pr[s][:], pr[s][:], mk_sb[:, mg, :]
                                        )
                                for s in (0, 1):
                                    h = 2 * hp + s
                                    for u in (0, 1):
                                        kt_i = 2 * g + u
                                        nc.tensor.matmul(
                                            po[s][0:65, :],
                                            r(V65[:, kt_i, ds(65 * h, 65)]),
                                            r(pr[s][:, ts(u, TQ)]),
                                            start=(kt_i == 0),
                                            stop=(kt_i == nkt - 1),
                                        )
                            # normalize + write attn^T
                            for s in (0, 1):
                                rcp = psm.tile([P, TQ], F32R, tag="rcp")
                                with nc.allow_low_precision(
                                    reason="f32r reciprocal is plenty for softmax norm"
                                ):
                                    nc.vector.reciprocal(
                                        rcp[64:65, :], po[s][64:65, :]
                                    )
                                pb = pbc.tile([P, TQ], F32, tag="bc")
                                nc.tensor.matmul(
                                    pb[0:64, :],
                                    r(ones[64:65, 0:64]),
                                    r(rcp[64:65, :]),
                                    start=True,
                                    stop=True,
                                )
                                pv_sb = psm.tile([64, TQ], F32, tag="tmp", name="pv_sb")
                                nc.vector.tensor_copy(pv_sb[:], po[s][0:64, :])
                                if s == 0:
                                    nc.vector.tensor_mul(
                                        attn[0:64, hp, ts(qi, TQ)],
                                        pv_sb[:],
                                        pb[0:64, :],
                                    )
                                else:
                                    tmp = psm.tile([64, TQ], F32R, tag="tmp")
                                    nc.vector.tensor_mul(
                                        tmp[:], pv_sb[:], pb[0:64, :]
                                    )
                                    nc.sync.dma_start(
                                        attn[64:128, hp, ts(qi, TQ)], tmp[:]
                                    )
                        # output projection for this q-range
                        for tsub in range(TQ // P):
                            tt = qi * (TQ // P) + tsub
                            for e2 in range(E // TQ):
                                pt = psc.tile([P, 1024], F32, tag="sc")
                                for co in range(CO):
                                    nc.tensor.matmul(
                                        pt[:, :TQ],
                                        r(attn[:, co, ds(tt * P, P)]),
                                        r(wo_sb[:, co, ts(e2, TQ)]),
                                        start=(co == 0),
                                        stop=(co == CO - 1),
                                    )
                                ob = psm.tile([P, TQ], F32, tag="ob", name="ob")
                                nc.vector.tensor_copy(ob[:], pt[:, :TQ])
                                nc.sync.dma_start(
                                    out_d[ds(tt * P, P), ts(e2, TQ)], ob[:]
                                )
    nc.compile()
    return nc


def build_null_nc():
    """Tiny kernel used to measure per-dispatch overhead."""
    nc = bacc.Bacc("TRN2", target_bir_lowering=False, debug=False)
    z_d = nc.dram_tensor("z", [1, 128], F32, kind="ExternalInput").ap()
    o_d = nc.dram_tensor("o", [1, 128], F32, kind="ExternalOutput").ap()
    with tile.TileContext(nc) as tc:
        with tc.tile_pool(name="sb", bufs=1) as sb:
            t = sb.tile([1, 128], F32, tag="t")
            nc.sync.dma_start(t[:], z_d)
            nc.sync.dma_start(o_d, t[:])
    nc.compile()
    return nc


# ---------------------------------------------------------------------------
# host side


def _masks():
    kk = np.arange(P)[:, None]
    qq = np.arange(TQ)[None, :]
    m = np.zeros((P, 2, 1024), dtype=np.float32)
    for rr in range(4):
        m[:, rr // 2, (rr % 2) * TQ : (rr % 2 + 1) * TQ] = (
            kk <= qq - P * rr
        ).astype(np.float32)
    return m


def _per_core_inputs(x, Wq, bq, Wk, bk, Wv, bv, Wo, T):
    """Build the 8 per-core input dicts (host-side slicing/layout)."""
    masks = _masks()
    in_maps = []
    for c in range(N_CORES):
        b, hg = c // 2, c % 2
        hs = slice(C * hg, C * (hg + 1))

        def to_pet(a, n_outer):  # [E_like, F] -> [P, n_outer, F]
            return np.ascontiguousarray(
                a.reshape(n_outer, P, a.shape[-1]).transpose(1, 0, 2)
            )

        xt = to_pet(np.ascontiguousarray(x[b].T.astype(np.float32)), EO)
        wq = to_pet(np.ascontiguousarray(Wq[hs].T.astype(np.float32)), EO)
        wk = to_pet(np.ascontiguousarray(Wk[hs].T.astype(np.float32)), EO)
        wv = to_pet(np.ascontiguousarray(Wv[hs].T.astype(np.float32)), EO)
        wo = to_pet(np.ascontiguousarray(Wo[:, hs].T.astype(np.float32)), CO)
        in_maps.append(
            {
                "xt": xt,
                "wq": wq,
                "wk": wk,
                "wv": wv,
                "wo": wo,
                "bq": np.ascontiguousarray(
                    bq[hs].astype(np.float32).reshape(CO, P).T
                ),
                "bk": np.ascontiguousarray(
                    bk[hs].astype(np.float32).reshape(CO, P).T
                ),
                "bvb": np.ascontiguousarray(
                    np.broadcast_to(bv[hs].astype(np.float32), (P, C))
                ),
                "masks": masks,
                "onesc": np.ones((P, 128), dtype=np.float32),
            }
        )
    return in_maps


_NC_CACHE = {}


def _get_nc(T, reps=1):
    key = (T, reps)
    if key not in _NC_CACHE:
        _NC_CACHE[key] = build_nc(T, reps)
    return _NC_CACHE[key]


def kernel(x, Wq, bq, Wk, bk, Wv, bv, Wo, bo):
    x = np.asarray(x, dtype=np.float32)
    B, T, _ = x.shape
    nc = _get_nc(T)
    in_maps = _per_core_inputs(
        x,
        np.asarray(Wq),
        np.asarray(bq),
        np.asarray(Wk),
        np.asarray(bk),
        np.asarray(Wv),
        np.asarray(bv),
        np.asarray(Wo),
        T,
    )
    res = run_bass_kernel_spmd(nc, in_maps, core_ids=list(range(N_CORES)))
    bo32 = np.asarray(bo, dtype=np.float32)
    out = np.empty((B, T, E), dtype=np.float32)
    for b in range(B):
        out[b] = res.results[2 * b]["out"] + res.results[2 * b + 1]["out"] + bo32
    return out
